# revision 6
# baseline (speedup 1.0000x reference)
"""Trainium2 Bass kernel for nn_Net_23210003267823 (BiGCN rumor-detection net).

Math (per branch, edge set A, weights W1,b1,W2,b2):
    U  = x @ W1                                  (big GEMM, memory-bound: x is 400 MB)
    Y  = D^-1/2 U ;  h1 = D^-1/2 (A Y + Y) + b1  (sym-normalized GCN conv w/ self loops)
    Q  = relu(x[root]) @ W2[64:]                 (root-extend folded: only 128 distinct root rows)
    z  = relu(h1) @ W2[:64] + Q[batch]
    h2 = relu(D^-1/2 (A Zt + Zt) + b2),  Zt = D^-1/2 z
    out_branch = [segment_mean(h2, batch) | h1[root] * (cnt>0)]
Final: log_softmax(concat(td, bu) @ fc_W + fc_b).

Sharding: nodes row-sharded over 8 cores (2500 real + 60 pad rows each).
AllGather of the 64+64-wide Y / Zt message tables (bf16); aggregation via
indirect-DMA row gather + is_equal one-hot matmul into PSUM, per 128-edge tile.
Host prep is integer index metadata only (edge partition/sort, degree counts).
"""
import sys, os
sys.path.insert(0, "/opt/trn_rl_repo")
import numpy as np

NC_ = 8
N, E, G = 20000, 320000, 128
IN, HID, OUT = 5000, 64, 64
RPC, PRC, NBLK = 2500, 2560, 20   # real rows/core, padded rows/core, row blocks
NPAD = NC_ * PRC                   # 20480
INP, NK = 5120, 40                 # padded IN, K blocks
BIG = np.float32(1e30)

_cache = {}


def _build(TB):
    KSTOP = int(os.environ.get("KSTOP", "99"))
    import concourse.bass as bass
    import concourse.mybir as mybir
    import concourse.tile as tile
    from concourse import bacc

    dt = mybir.dt
    f32, bf16, i32 = dt.float32, dt.bfloat16, dt.int32
    AF = mybir.ActivationFunctionType
    OP = mybir.AluOpType

    nc = bacc.Bacc("TRN2", target_bir_lowering=False, debug=False, num_devices=NC_)

    # ---------------- I/O ----------------
    xc = nc.dram_tensor("xc", [RPC, IN], f32, kind="ExternalInput")
    w1 = nc.dram_tensor("w1", [IN, 128], f32, kind="ExternalInput")
    w2a = nc.dram_tensor("w2a", [128, 128], f32, kind="ExternalInput")
    w2b = nc.dram_tensor("w2b", [IN, 128], f32, kind="ExternalInput")
    bias1 = nc.dram_tensor("bias1", [128, 128], f32, kind="ExternalInput")
    bias2 = nc.dram_tensor("bias2", [128, 128], f32, kind="ExternalInput")
    deg = nc.dram_tensor("deg", [2, PRC], f32, kind="ExternalInput")
    srcs = nc.dram_tensor("srcs", [2, NBLK, 128, TB], i32, kind="ExternalInput")
    drel = nc.dram_tensor("drel", [2, NBLK, 128, TB], f32, kind="ExternalInput")
    brel = nc.dram_tensor("brel", [PRC], f32, kind="ExternalInput")
    bidx = nc.dram_tensor("bidx", [PRC], i32, kind="ExternalInput")
    rloc = nc.dram_tensor("rloc", [G], i32, kind="ExternalInput")
    rxloc = nc.dram_tensor("rxloc", [G], i32, kind="ExternalInput")
    fcw = nc.dram_tensor("fcw", [2, 128, 256], f32, kind="ExternalInput")
    fcb = nc.dram_tensor("fcb", [128, 2], f32, kind="ExternalInput")
    out = nc.dram_tensor("out", [G, 2], f32, kind="ExternalOutput")

    # ---------------- internal DRAM ----------------
    Ylocal = nc.dram_tensor("Ylocal", [PRC, 128], bf16)
    Yfull = nc.dram_tensor("Yfull", [NPAD, 128], bf16, addr_space="Shared")
    Ztlocal = nc.dram_tensor("Ztlocal", [PRC, 128], bf16)
    Ztfull = nc.dram_tensor("Ztfull", [NPAD, 128], bf16, addr_space="Shared")
    h1loc = nc.dram_tensor("h1loc", [PRC + 1, 128], f32)
    Qtab = nc.dram_tensor("Qtab", [G + 1, 128], f32, addr_space="Shared")
    qbl = nc.dram_tensor("qbl", [G, 128], f32)
    arl = nc.dram_tensor("arl", [128, 257], f32)
    arf = nc.dram_tensor("arf", [128, 257], f32, addr_space="Shared")

    RG = [list(range(NC_))]

    with tile.TileContext(nc) as tc:
        with tc.tile_pool(name="const", bufs=1) as cp:
            # iota row 0..127 on every partition (f32)
            ii = cp.tile([128, 128], i32)
            nc.gpsimd.iota(ii[:], pattern=[[1, 128]], base=0, channel_multiplier=0)
            iof = cp.tile([128, 128], f32)
            nc.vector.tensor_copy(iof[:], ii[:])

            # dinv [128, 40]: col br*NBLK+blk
            dga = cp.tile([128, NBLK * 2], f32)
            nc.sync.dma_start(out=dga[:], in_=deg[:].rearrange("t (b p) -> p (t b)", p=128))
            drc = cp.tile([128, NBLK * 2], f32)
            nc.vector.reciprocal(drc[:], dga[:])
            dinv = cp.tile([128, NBLK * 2], f32)
            nc.scalar.activation(dinv[:], drc[:], AF.Sqrt)

            b1t = cp.tile([128, 128], f32)
            nc.sync.dma_start(out=b1t[:], in_=bias1[:])
            b2t = cp.tile([128, 128], f32)
            nc.sync.dma_start(out=b2t[:], in_=bias2[:])
            w2at = cp.tile([128, 128], bf16)
            nc.gpsimd.dma_start(out=w2at[:], in_=w2a[:])
            brelt = cp.tile([128, NBLK], f32)
            nc.sync.dma_start(out=brelt[:], in_=brel[:].rearrange("(b p) -> p b", p=128))
            bidxt = cp.tile([128, NBLK], i32)
            nc.sync.dma_start(out=bidxt[:], in_=bidx[:].rearrange("(b p) -> p b", p=128))
            rloct = cp.tile([128, 1], i32)
            nc.sync.dma_start(out=rloct[:], in_=rloc[:, None])
            rxloct = cp.tile([128, 1], i32)
            nc.sync.dma_start(out=rxloct[:], in_=rxloc[:, None])
            fcw0 = cp.tile([128, 256], f32)
            nc.sync.dma_start(out=fcw0[:], in_=fcw[0])
            fcw1 = cp.tile([128, 256], f32)
            nc.sync.dma_start(out=fcw1[:], in_=fcw[1])
            fcbt = cp.tile([128, 2], f32)
            nc.sync.dma_start(out=fcbt[:], in_=fcb[:])

            # zero rows for h1loc[2560] and Qtab[128]
            zrow = cp.tile([1, 128], f32)
            nc.vector.memset(zrow[:], 0.0)
            nc.sync.dma_start(out=h1loc[PRC:PRC + 1, :], in_=zrow[:])
            nc.sync.dma_start(out=Qtab[G:G + 1, :], in_=zrow[:])

            # ---------------- phase R: root rows -> Q (partial) ----------------
            if KSTOP >= 1:
             with tc.tile_pool(name="pr", bufs=2) as pr, \
                 tc.tile_pool(name="prp", bufs=1, space="PSUM") as prp:
                Rt_ = pr.tile([128, INP], f32, tag="rbig")
                nc.vector.memset(Rt_[:], 0.0)
                nc.gpsimd.indirect_dma_start(
                    out=Rt_[:, 0:IN], out_offset=None, in_=xc[:],
                    in_offset=bass.IndirectOffsetOnAxis(ap=rxloct[:, :1], axis=0),
                    bounds_check=RPC - 1, oob_is_err=False)
                Rr = pr.tile([128, INP], bf16, tag="rbig2")
                nc.scalar.activation(Rr[:], Rt_[:], AF.Relu)
                pq = prp.tile([128, 128], f32)
                for k in range(NK):
                    rtk = pr.tile([128, 128], bf16, tag="rtk")
                    nc.sync.dma_start(out=rtk[:], in_=Rr[:, k * 128:(k + 1) * 128], transpose=True)
                    wbk = pr.tile([128, 128], bf16, tag="wbk")
                    if k == NK - 1:
                        nc.vector.memset(wbk[:], 0.0)
                        nc.gpsimd.dma_start(out=wbk[0:IN - 128 * k, :], in_=w2b[128 * k:IN, :])
                    else:
                        nc.gpsimd.dma_start(out=wbk[:], in_=w2b[128 * k:128 * (k + 1), :])
                    nc.tensor.matmul(out=pq[:], lhsT=rtk[:], rhs=wbk[:], start=(k == 0), stop=(k == NK - 1))
                qsb = pr.tile([128, 128], f32, tag="qsb")
                nc.vector.tensor_copy(qsb[:], pq[:])
                nc.sync.dma_start(out=qbl[:], in_=qsb[:])
            if KSTOP >= 1:
             nc.gpsimd.collective_compute("AllReduce", OP.add, replica_groups=RG,
                                          ins=[qbl[:]], outs=[Qtab[0:G, :]])

            # ---------------- phase G: U^T = W1^T x^T ; Y ----------------
            if KSTOP >= 2:
             with tc.tile_pool(name="pw", bufs=1) as pw, \
                 tc.tile_pool(name="px", bufs=5) as px, \
                 tc.tile_pool(name="pxt", bufs=3) as pxt, \
                 tc.tile_pool(name="pub", bufs=3) as pub, \
                 tc.tile_pool(name="pup", bufs=2, space="PSUM") as pup:
                w1all = pw.tile([128, NK * 128], bf16)
                nc.vector.memset(w1all[:, 39 * 128:], 0.0)
                nc.gpsimd.dma_start(out=w1all[:, 0:39 * 128].rearrange("p (k f) -> p k f", f=128),
                                    in_=w1[0:4992, :].rearrange("(k p) f -> p k f", p=128))
                nc.gpsimd.dma_start(out=w1all[0:8, 39 * 128:40 * 128], in_=w1[4992:IN, :])

                for rc in range(5):
                    xbs = []
                    for j in range(4):
                        bi = rc * 4 + j
                        row0 = bi * 128
                        nr = min(128, RPC - row0)
                        xb = px.tile([128, INP], bf16, tag="xb")
                        if nr < 128:
                            nc.vector.memset(xb[:], 0.0)
                        else:
                            nc.vector.memset(xb[:, IN:INP], 0.0)
                        nc.gpsimd.dma_start(out=xb[0:nr, 0:IN], in_=xc[row0:row0 + nr, :])
                        xbs.append(xb)
                    pu = pup.tile([128, 512], f32)
                    for k in range(NK):
                        xt = pxt.tile([128, 512], bf16, tag="xt")
                        for j in range(4):
                            nc.sync.dma_start(out=xt[:, j * 128:(j + 1) * 128],
                                              in_=xbs[j][:, k * 128:(k + 1) * 128], transpose=True)
                        nc.tensor.matmul(out=pu[:], lhsT=w1all[:, k * 128:(k + 1) * 128], rhs=xt[:],
                                         start=(k == 0), stop=(k == NK - 1))
                    ut = pub.tile([128, 512], bf16, tag="ut")
                    nc.vector.tensor_copy(ut[:], pu[:])
                    for j in range(4):
                        bi = rc * 4 + j
                        ub = pub.tile([128, 128], bf16, tag="ub")
                        nc.sync.dma_start(out=ub[:], in_=ut[:, j * 128:(j + 1) * 128], transpose=True)
                        yb = pub.tile([128, 128], bf16, tag="yb")
                        nc.vector.tensor_scalar(out=yb[:, 0:64], in0=ub[:, 0:64],
                                                scalar1=dinv[:, bi:bi + 1], scalar2=None, op0=OP.mult)
                        nc.vector.tensor_scalar(out=yb[:, 64:128], in0=ub[:, 64:128],
                                                scalar1=dinv[:, NBLK + bi:NBLK + bi + 1], scalar2=None, op0=OP.mult)
                        nc.sync.dma_start(out=Ylocal[bi * 128:(bi + 1) * 128, :], in_=yb[:])

            if KSTOP >= 3:
             nc.gpsimd.collective_compute("AllGather", OP.bypass, replica_groups=RG,
                                          ins=[Ylocal[:]], outs=[Yfull[:]])

            # ---------------- conv helper ----------------
            def agg_block(pools, table, blk, br, TBn):
                """accumulate A@table for dst block blk, branch br -> psum tile [128,64]"""
                pa, pv, po, ph = pools
                st = pa.tile([128, TBn], i32, tag="st")
                nc.sync.dma_start(out=st[:], in_=srcs[br, blk])
                dr_ = pa.tile([128, TBn], f32, tag="dr")
                nc.sync.dma_start(out=dr_[:], in_=drel[br, blk])
                ph_ = ph.tile([128, 64], f32)
                for t in range(TBn):
                    V = pv.tile([128, 64], bf16, tag="v")
                    nc.gpsimd.indirect_dma_start(
                        out=V[:], out_offset=None, in_=table[:],
                        in_offset=bass.IndirectOffsetOnAxis(ap=st[:, t:t + 1], axis=0),
                        element_offset=64 * br)
                    oh = po.tile([128, 128], bf16, tag="oh")
                    nc.vector.tensor_tensor(out=oh[:], in0=dr_[:, t:t + 1].to_broadcast([128, 128]),
                                            in1=iof[:], op=OP.is_equal)
                    nc.tensor.matmul(out=ph_[:], lhsT=oh[:], rhs=V[:], start=(t == 0), stop=(t == TBn - 1))
                return ph_

            # ---------------- phase C1: conv1 -> h1, z, Zt ----------------
            if KSTOP >= 4:
             with tc.tile_pool(name="pa1", bufs=3) as pa, \
                 tc.tile_pool(name="pv1", bufs=6) as pv, \
                 tc.tile_pool(name="po1", bufs=6) as po, \
                 tc.tile_pool(name="pm1", bufs=3) as pm, \
                 tc.tile_pool(name="ph1", bufs=2, space="PSUM") as ph, \
                 tc.tile_pool(name="pz1", bufs=2, space="PSUM") as pz:
                pools = (pa, pv, po, ph)
                for blk in range(NBLK):
                    h1f = pm.tile([128, 128], f32, tag="h1f")
                    h1b = pm.tile([128, 128], bf16, tag="h1b")
                    for br in range(2):
                        ph_ = agg_block(pools, Yfull, blk, br, TB)
                        ys = pm.tile([128, 64], bf16, tag="ys")
                        nc.sync.dma_start(out=ys[:], in_=Ylocal[blk * 128:(blk + 1) * 128, br * 64:(br + 1) * 64])
                        hs = pm.tile([128, 64], f32, tag="hs")
                        nc.vector.tensor_tensor(out=hs[:], in0=ph_[:], in1=ys[:], op=OP.add)
                        nc.vector.tensor_scalar(out=hs[:], in0=hs[:],
                                                scalar1=dinv[:, br * NBLK + blk:br * NBLK + blk + 1],
                                                scalar2=None, op0=OP.mult)
                        nc.vector.tensor_tensor(out=h1f[:, br * 64:(br + 1) * 64], in0=hs[:],
                                                in1=b1t[:, br * 64:(br + 1) * 64], op=OP.add)
                        nc.vector.tensor_tensor(out=h1b[:, br * 64:(br + 1) * 64], in0=hs[:],
                                                in1=b1t[:, br * 64:(br + 1) * 64], op=OP.add)
                    nc.sync.dma_start(out=h1loc[blk * 128:(blk + 1) * 128, :], in_=h1f[:])
                    hr = pm.tile([128, 128], bf16, tag="hr")
                    nc.scalar.activation(hr[:], h1b[:], AF.Relu)
                    hrT = pm.tile([128, 128], bf16, tag="hrT")
                    nc.sync.dma_start(out=hrT[:], in_=hr[:], transpose=True)
                    pz_ = pz.tile([128, 128], f32)
                    nc.tensor.matmul(out=pz_[:], lhsT=hrT[:], rhs=w2at[:], start=True, stop=True)
                    qg = pm.tile([128, 128], f32, tag="qg")
                    nc.gpsimd.indirect_dma_start(
                        out=qg[:], out_offset=None, in_=Qtab[:],
                        in_offset=bass.IndirectOffsetOnAxis(ap=bidxt[:, blk:blk + 1], axis=0))
                    zf = pm.tile([128, 128], f32, tag="zf")
                    nc.vector.tensor_tensor(out=zf[:], in0=pz_[:], in1=qg[:], op=OP.add)
                    ztb = pm.tile([128, 128], bf16, tag="ztb")
                    nc.vector.tensor_scalar(out=ztb[:, 0:64], in0=zf[:, 0:64],
                                            scalar1=dinv[:, blk:blk + 1], scalar2=None, op0=OP.mult)
                    nc.vector.tensor_scalar(out=ztb[:, 64:128], in0=zf[:, 64:128],
                                            scalar1=dinv[:, NBLK + blk:NBLK + blk + 1], scalar2=None, op0=OP.mult)
                    nc.sync.dma_start(out=Ztlocal[blk * 128:(blk + 1) * 128, :], in_=ztb[:])

            if KSTOP >= 5:
             nc.gpsimd.collective_compute("AllGather", OP.bypass, replica_groups=RG,
                                          ins=[Ztlocal[:]], outs=[Ztfull[:]])

            # ---------------- phase C2: conv2 -> h2 -> segment sums ----------------
            if KSTOP >= 6:
             with tc.tile_pool(name="pa2", bufs=3) as pa2, \
                 tc.tile_pool(name="pv2", bufs=6) as pv2, \
                 tc.tile_pool(name="po2", bufs=6) as po2, \
                 tc.tile_pool(name="pm2", bufs=3) as pm2, \
                 tc.tile_pool(name="ph2", bufs=2, space="PSUM") as ph2, \
                 tc.tile_pool(name="ps2", bufs=1, space="PSUM") as ps2:
                pools2 = (pa2, pv2, po2, ph2)
                pseg = ps2.tile([128, 129], f32)
                for blk in range(NBLK):
                    pay = pm2.tile([128, 129], f32, tag="pay")
                    nc.vector.memset(pay[:, 128:129], 1.0)
                    for br in range(2):
                        ph_ = agg_block(pools2, Ztfull, blk, br, TB)
                        zs = pm2.tile([128, 64], bf16, tag="zs")
                        nc.sync.dma_start(out=zs[:], in_=Ztlocal[blk * 128:(blk + 1) * 128, br * 64:(br + 1) * 64])
                        hs2 = pm2.tile([128, 64], f32, tag="hs2")
                        nc.vector.tensor_tensor(out=hs2[:], in0=ph_[:], in1=zs[:], op=OP.add)
                        nc.vector.tensor_scalar(out=hs2[:], in0=hs2[:],
                                                scalar1=dinv[:, br * NBLK + blk:br * NBLK + blk + 1],
                                                scalar2=None, op0=OP.mult)
                        nc.vector.tensor_tensor(out=hs2[:], in0=hs2[:],
                                                in1=b2t[:, br * 64:(br + 1) * 64], op=OP.add)
                        nc.scalar.activation(pay[:, br * 64:(br + 1) * 64], hs2[:], AF.Relu)
                    ohs = pm2.tile([128, 128], f32, tag="ohs")
                    nc.vector.tensor_tensor(out=ohs[:], in0=brelt[:, blk:blk + 1].to_broadcast([128, 128]),
                                            in1=iof[:], op=OP.is_equal)
                    nc.tensor.matmul(out=pseg[:], lhsT=ohs[:], rhs=pay[:], start=(blk == 0), stop=(blk == NBLK - 1))

                rg = pm2.tile([128, 128], f32, tag="rg")
                nc.gpsimd.indirect_dma_start(
                    out=rg[:], out_offset=None, in_=h1loc[:],
                    in_offset=bass.IndirectOffsetOnAxis(ap=rloct[:, :1], axis=0))
                part = pm2.tile([128, 257], f32, tag="part")
                nc.vector.tensor_copy(part[:, 0:129], pseg[:])
                nc.vector.tensor_copy(part[:, 129:257], rg[:])
                nc.sync.dma_start(out=arl[:], in_=part[:])

            if KSTOP >= 7:
             nc.gpsimd.collective_compute("AllReduce", OP.add, replica_groups=RG,
                                          ins=[arl[:]], outs=[arf[:]])

            # ---------------- final ----------------
            if KSTOP >= 7:
             with tc.tile_pool(name="pf", bufs=1) as pf:
                Rt = pf.tile([128, 257], f32)
                nc.sync.dma_start(out=Rt[:], in_=arf[:])
                cnt = Rt[:, 128:129]
                c1 = pf.tile([128, 1], f32)
                nc.vector.tensor_scalar_max(out=c1[:], in0=cnt, scalar1=1.0)
                rec = pf.tile([128, 1], f32)
                nc.vector.reciprocal(rec[:], c1[:])
                ind = pf.tile([128, 1], f32)
                nc.vector.tensor_scalar_min(out=ind[:], in0=cnt, scalar1=1.0)
                hfc = pf.tile([128, 256], f32)
                nc.vector.tensor_scalar(out=hfc[:, 0:64], in0=Rt[:, 0:64], scalar1=rec[:, :1], scalar2=None, op0=OP.mult)
                nc.vector.tensor_scalar(out=hfc[:, 64:128], in0=Rt[:, 129:193], scalar1=ind[:, :1], scalar2=None, op0=OP.mult)
                nc.vector.tensor_scalar(out=hfc[:, 128:192], in0=Rt[:, 64:128], scalar1=rec[:, :1], scalar2=None, op0=OP.mult)
                nc.vector.tensor_scalar(out=hfc[:, 192:256], in0=Rt[:, 193:257], scalar1=ind[:, :1], scalar2=None, op0=OP.mult)
                lg = pf.tile([128, 2], f32)
                for j, fw in enumerate((fcw0, fcw1)):
                    tmp = pf.tile([128, 256], f32, tag=f"tmp{j}")
                    nc.vector.tensor_tensor(out=tmp[:], in0=hfc[:], in1=fw[:], op=OP.mult)
                    nc.vector.reduce_sum(lg[:, j:j + 1], tmp[:], axis=mybir.AxisListType.X)
                nc.vector.tensor_tensor(out=lg[:], in0=lg[:], in1=fcbt[:], op=OP.add)
                mx = pf.tile([128, 1], f32)
                nc.vector.reduce_max(mx[:], lg[:], axis=mybir.AxisListType.X)
                d_ = pf.tile([128, 2], f32)
                nc.vector.tensor_scalar(out=d_[:], in0=lg[:], scalar1=mx[:, :1], scalar2=None, op0=OP.subtract)
                e_ = pf.tile([128, 2], f32)
                nc.scalar.activation(e_[:], d_[:], AF.Exp)
                s_ = pf.tile([128, 1], f32)
                nc.vector.reduce_sum(s_[:], e_[:], axis=mybir.AxisListType.X)
                ls = pf.tile([128, 1], f32)
                nc.scalar.activation(ls[:], s_[:], AF.Ln)
                ov = pf.tile([128, 2], f32)
                nc.vector.tensor_scalar(out=ov[:], in0=d_[:], scalar1=ls[:, :1], scalar2=None, op0=OP.subtract)
                nc.sync.dma_start(out=out[:], in_=ov[:])

    nc.compile()
    return nc


def _prep(x, edge_index, bu_edge_index, batch, root_index,
          W1_td, b1_td, W2_td, b2_td, W1_bu, b1_bu, W2_bu, b2_bu, fc_W, fc_b):
    """Host-side: integer index metadata + parameter reshaping (no float math on data)."""
    x = np.asarray(x, np.float32)
    batch = np.asarray(batch).astype(np.int64)
    root_index = np.asarray(root_index).astype(np.int64)
    edges = [np.asarray(edge_index).astype(np.int64), np.asarray(bu_edge_index).astype(np.int64)]

    # per-(branch) degree over REAL node ids (dst count + self loop)
    degs = []
    for ei in edges:
        d = np.bincount(ei[1], minlength=N).astype(np.int64) + 1
        degs.append(d)

    # block edge counts -> shared TB
    maxcnt = 0
    blk_edges = [[[None] * NBLK for _ in range(2)] for _ in range(NC_)]
    for br, ei in enumerate(edges):
        src, dst = ei[0], ei[1]
        c = dst // RPC
        loc = dst - c * RPC
        blk = loc // 128
        rel = loc - blk * 128
        ps = (src // RPC) * PRC + (src - (src // RPC) * RPC)
        key = c * NBLK + blk
        order = np.argsort(key, kind="stable")
        ks = key[order]
        bounds = np.searchsorted(ks, np.arange(NC_ * NBLK + 1))
        for c_ in range(NC_):
            for b_ in range(NBLK):
                sl = order[bounds[c_ * NBLK + b_]:bounds[c_ * NBLK + b_ + 1]]
                blk_edges[c_][br][b_] = (ps[sl], rel[sl])
                maxcnt = max(maxcnt, len(sl))
    TB = max(1, (maxcnt + 127) // 128)

    srcs = np.zeros((NC_, 2, NBLK, 128, TB), np.int32)
    drel = np.full((NC_, 2, NBLK, 128, TB), -1.0, np.float32)
    for c in range(NC_):
        for br in range(2):
            for b in range(NBLK):
                s, r = blk_edges[c][br][b]
                n = len(s)
                lane, til = np.arange(n) % 128, np.arange(n) // 128
                srcs[c, br, b, lane, til] = s
                drel[c, br, b, lane, til] = r

    deg = np.full((NC_, 2, PRC), BIG, np.float32)
    for br in range(2):
        deg[:, br, :RPC] = degs[br].reshape(NC_, RPC).astype(np.float32)

    brel = np.full((NC_, PRC), -1.0, np.float32)
    brel[:, :RPC] = batch.reshape(NC_, RPC).astype(np.float32)
    bidx = np.full((NC_, PRC), G, np.int32)
    bidx[:, :RPC] = batch.reshape(NC_, RPC).astype(np.int32)

    rc = root_index // RPC
    rl = root_index - rc * RPC
    rloc = np.full((NC_, G), PRC, np.int32)
    rxloc = np.full((NC_, G), 1 << 20, np.int32)
    for g in range(G):
        rloc[rc[g], g] = rl[g]
        rxloc[rc[g], g] = rl[g]

    # parameters (pure reshapes / replication)
    w1 = np.hstack([np.asarray(W1_td, np.float32), np.asarray(W1_bu, np.float32)])        # [5000,128]
    w2a = np.zeros((128, 128), np.float32)  # block-diag: one K=128 matmul covers both branches
    w2a[0:64, 0:64] = np.asarray(W2_td, np.float32)[:HID]
    w2a[64:128, 64:128] = np.asarray(W2_bu, np.float32)[:HID]
    w2b = np.hstack([np.asarray(W2_td, np.float32)[HID:], np.asarray(W2_bu, np.float32)[HID:]])  # [5000,128]
    bias1 = np.broadcast_to(np.concatenate([np.asarray(b1_td, np.float32), np.asarray(b1_bu, np.float32)]), (128, 128)).copy()
    bias2 = np.broadcast_to(np.concatenate([np.asarray(b2_td, np.float32), np.asarray(b2_bu, np.float32)]), (128, 128)).copy()
    fcw = np.stack([np.broadcast_to(np.asarray(fc_W, np.float32)[:, j], (128, 256)) for j in range(2)])
    fcb = np.broadcast_to(np.asarray(fc_b, np.float32), (128, 2)).copy()

    in_maps = []
    for c in range(NC_):
        in_maps.append(dict(
            xc=np.ascontiguousarray(x[c * RPC:(c + 1) * RPC]),
            w1=w1, w2a=w2a, w2b=w2b, bias1=bias1, bias2=bias2,
            deg=np.ascontiguousarray(deg[c]),
            srcs=np.ascontiguousarray(srcs[c]), drel=np.ascontiguousarray(drel[c]),
            brel=np.ascontiguousarray(brel[c]), bidx=np.ascontiguousarray(bidx[c]),
            rloc=np.ascontiguousarray(rloc[c]), rxloc=np.ascontiguousarray(rxloc[c]),
            fcw=np.ascontiguousarray(fcw), fcb=fcb,
        ))
    return TB, in_maps


def kernel(**inputs):
    from concourse.bass_utils import run_bass_kernel_spmd
    TB, in_maps = _prep(**inputs)
    if TB not in _cache:
        _cache[TB] = _build(TB)
    nc = _cache[TB]
    res = run_bass_kernel_spmd(nc, in_maps, list(range(NC_)))
    return res.results[0]["out"]


if __name__ == "__main__":
    import reference
    inputs = {k: np.asarray(v) for k, v in reference.setup_inputs().items()}
    got = kernel(**inputs)
    print(got[:4])
